# revision 28
# baseline (speedup 1.0000x reference)
"""Trainium2 Bass kernel for nn_EphysAttentionLayer.

Reference semantics:
    s  = spikes.f32                              # [B, N] in {0,1}
    PD = exp(-DT / exp(tau_pre))                 # [N, N]
    QD = exp(-DT / exp(tau_post))
    pt' = pt*PD + s[b,j]*exp(w_pre)*DT
    qt' = qt*QD + s[b,i]*exp(w_post)*DT
    A'  = clip(att + (1-att)*pt'*si - att*qt'*sj, -0.5, 1.5)
    out = A' @ v_w.T + v_b                       # [B, N, E]

Sharding: rows (post-synaptic axis i) split across 8 cores, 128 rows each.
Per-core layout: [i on partitions, j in free dim], one batch at a time.

Key structure (per batch):
  u  = si * (PD*pt + SJ*preW')        (preW' = exp(w_pre + ln DT))
  q' = QD*qt + si*postW'              (post trace update)
  w  = u + SJ*q'
  d  = u - att*w                      (small delta, bf16)
  x  = att + d                        (accumulated transposed in PSUM via
                                       identity matmuls: x.T = att.T + d.T)
  A' = clip(x) = 1.5 - y2,  y2 = relu(2 - relu(x + 0.5))   (two ACT passes)
  out = y2 @ (-v_w.T) + (v_b + 1.5*rowsum(v_w))            (bias via K=1 MM)

dtypes: traces bf16, att fp16, latents bf16, compute chain bf16, x in fp32
PSUM; the output matmul runs in float32r (fast fp32 streaming mode).
The SJ broadcast masks and packed inputs are prepared host-side as part of
sharding; all O(N^2) compute stays on device.
"""

import math

import numpy as np
import ml_dtypes

import concourse.bacc as bacc
import concourse.mybir as mybir
import concourse.tile as tile
from concourse.bass_utils import run_bass_kernel_spmd

B, N, E = 8, 1024, 512
NCORES = 8
R = N // NCORES  # 128 rows per core
JC = N // 128    # 8 column chunks
DT = 0.001
LN_DT = math.log(DT)
MIN_ATTN, MAX_ATTN = -0.5, 1.5

F32 = mybir.dt.float32
F32R = mybir.dt.float32r
BF16 = mybir.dt.bfloat16
FP16 = mybir.dt.float16
AOP = mybir.AluOpType
AFT = mybir.ActivationFunctionType

_BUILD_CACHE = {}


def _build_nc():
    # Bacc (not raw Bass): its compile pipeline splits multi-sem waits into
    # InstEventSemaphore chains, which walrus codegen requires on TRN2.
    nc = bacc.Bacc()

    # pk: per-batch packed [pt | qt | SJ] along the free dim, bf16
    pk_d = nc.declare_dram_parameter("pk", [B, R, 3 * N], BF16, isOutput=False)
    att_d = nc.declare_dram_parameter("att", [B, R, N], FP16, isOutput=False)
    # lat: packed [tau_pre | tau_post | w_pre | w_post], bf16
    lat_d = nc.declare_dram_parameter("lat", [R, 4 * N], BF16, isOutput=False)
    si_d = nc.declare_dram_parameter("si", [R, B], F32, isOutput=False)
    vwTn_d = nc.declare_dram_parameter("vwTn", [N, E], F32R, isOutput=False)
    vb_d = nc.declare_dram_parameter("vb", [1, E], F32R, isOutput=False)
    ones_d = nc.declare_dram_parameter("ones", [1, 128], F32R, isOutput=False)
    idf_d = nc.declare_dram_parameter("idf", [128, 128], FP16, isOutput=False)
    idb_d = nc.declare_dram_parameter("idb", [128, 128], BF16, isOutput=False)
    idbn_d = nc.declare_dram_parameter("idbn", [128, 128], BF16, isOutput=False)
    out_d = nc.declare_dram_parameter("out", [B, R, E], F32, isOutput=True)

    with tile.TileContext(nc) as tc:
        with (
            tc.sbuf_pool(name="const", bufs=1) as cpool,
            tc.sbuf_pool(name="work", bufs=2) as wpool,
            tc.psum_pool(name="pxt_pool", bufs=3) as pp_xt,
            tc.psum_pool(name="po_pool", bufs=2) as pp_o,
        ):
            # ---- constants ----
            lndt_col = cpool.tile([128, 1], F32)
            nc.vector.memset(lndt_col[:, :], LN_DT)
            half_col = cpool.tile([128, 1], F32)
            nc.vector.memset(half_col[:, :], 0.5)
            two_col = cpool.tile([128, 1], F32)
            nc.vector.memset(two_col[:, :], 2.0)

            lat_sb = cpool.tile([R, 4 * N], BF16)
            nc.sync.dma_start(lat_sb[:, :], lat_d[:, :])
            tau_pre = lat_sb[:, 0 * N:1 * N]
            tau_post = lat_sb[:, 1 * N:2 * N]
            w_pre = lat_sb[:, 2 * N:3 * N]
            w_post = lat_sb[:, 3 * N:4 * N]

            # e1 = exp(LN_DT - tau) = DT/exp(tau)  (ACT, one pass per tau)
            # PD = exp(-e1) ~= 1 - e1 + e1^2/2  (DVE Taylor; |e1|<=0.15 for
            # tau >= -5, so the cubic term <= 6e-4 -- below bf16 noise)
            e1p = cpool.tile([R, N], BF16)
            e1q = cpool.tile([R, N], BF16)
            PD = cpool.tile([R, N], BF16)
            QD = cpool.tile([R, N], BF16)
            preW = cpool.tile([R, N], BF16)
            postW = cpool.tile([R, N], BF16)
            nc.scalar.activation(e1p[:, :], tau_pre, AFT.Exp,
                                 bias=lndt_col[:, :], scale=-1.0)
            nc.scalar.activation(e1q[:, :], tau_post, AFT.Exp,
                                 bias=lndt_col[:, :], scale=-1.0)
            nc.scalar.activation(preW[:, :], w_pre, AFT.Exp,
                                 bias=lndt_col[:, :], scale=1.0)
            nc.scalar.activation(postW[:, :], w_post, AFT.Exp,
                                 bias=lndt_col[:, :], scale=1.0)
            gp = cpool.tile([R, N], BF16)
            gq = cpool.tile([R, N], BF16)
            nc.vector.tensor_scalar(gp[:, :], e1p[:, :], -0.5, 1.0, AOP.mult, AOP.add)
            nc.vector.tensor_scalar(gq[:, :], e1q[:, :], -0.5, 1.0, AOP.mult, AOP.add)
            nc.vector.tensor_mul(PD[:, :], e1p[:, :], gp[:, :])
            nc.vector.tensor_mul(QD[:, :], e1q[:, :], gq[:, :])
            nc.vector.tensor_scalar(PD[:, :], PD[:, :], -1.0, 1.0, AOP.mult, AOP.add)
            nc.vector.tensor_scalar(QD[:, :], QD[:, :], -1.0, 1.0, AOP.mult, AOP.add)

            si_sb = cpool.tile([R, B], F32)
            nc.sync.dma_start(si_sb[:, :], si_d[:, :])
            idf = cpool.tile([128, 128], FP16)
            nc.sync.dma_start(idf[:, :], idf_d[:, :])
            idb = cpool.tile([128, 128], BF16)
            nc.sync.dma_start(idb[:, :], idb_d[:, :])
            idbn = cpool.tile([128, 128], BF16)
            nc.sync.dma_start(idbn[:, :], idbn_d[:, :])
            vb_sb = cpool.tile([1, E], F32R)
            nc.sync.dma_start(vb_sb[:, :], vb_d[:, :])
            ones = cpool.tile([1, 128], F32R)
            nc.sync.dma_start(ones[:, :], ones_d[:, :])
            # vwTn DMA last: it is only needed by the first out-matmul (~15us
            # in) and must not delay the first batches' input DMAs.
            vwTn = cpool.tile([128, JC * E], F32R)  # chunk jc at [:, jc*E:(jc+1)*E]
            for jc in range(JC):
                nc.sync.dma_start(vwTn[:, jc * E:(jc + 1) * E],
                                  vwTn_d[jc * 128:(jc + 1) * 128, :])

            # ---- phase B: per-batch pipeline ----
            # Emitted as generators interleaved in pairs: consecutive DVE/ACT
            # instructions come from different batches, hiding the per-op
            # write-ack latency that would otherwise bubble dependent chains.

            def batch_chain(b):
                pk = wpool.tile([R, 3 * N], BF16, tag="pk", bufs=4, name=f"pk{b}")
                att = wpool.tile([R, N], FP16, tag="att", bufs=6, name=f"att{b}")
                nc.gpsimd.dma_start(pk[:, :], pk_d[b, :, :])
                nc.gpsimd.dma_start(att[:, :], att_d[b, :, :])
                pt = pk[:, 0 * N:1 * N]
                qt = pk[:, 1 * N:2 * N]
                SJ = pk[:, 2 * N:3 * N]
                si_b = si_sb[:, b:b + 1]
                yield

                # independent products first (DVE, bf16 2x)
                c1 = wpool.tile([R, N], BF16, tag="c1", name=f"c1{b}")
                nc.vector.tensor_mul(c1[:, :], PD[:, :], pt)
                yield
                m2 = wpool.tile([R, N], BF16, tag="m2", name=f"m2{b}")
                nc.vector.tensor_mul(m2[:, :], SJ, preW[:, :])
                yield
                a2 = wpool.tile([R, N], BF16, tag="a2", name=f"a2{b}")
                nc.vector.tensor_mul(a2[:, :], QD[:, :], qt)
                yield
                m3 = wpool.tile([R, N], BF16, tag="m3", name=f"m3{b}")
                nc.vector.tensor_scalar_mul(m3[:, :], postW[:, :], si_b)
                yield
                u0 = wpool.tile([R, N], BF16, tag="u0", name=f"u0{b}")
                nc.vector.tensor_add(u0[:, :], c1[:, :], m2[:, :])
                yield
                v0 = wpool.tile([R, N], BF16, tag="v0", name=f"v0{b}")
                nc.vector.tensor_add(v0[:, :], a2[:, :], m3[:, :])
                yield
                u = wpool.tile([R, N], BF16, tag="u", bufs=8, name=f"u{b}")
                nc.vector.tensor_scalar_mul(u[:, :], u0[:, :], si_b)
                yield
                vv = wpool.tile([R, N], BF16, tag="vv", name=f"vv{b}")
                nc.vector.tensor_mul(vv[:, :], SJ, v0[:, :])
                yield
                w = wpool.tile([R, N], BF16, tag="w", name=f"w{b}")
                nc.vector.tensor_add(w[:, :], u[:, :], vv[:, :])
                yield
                # tt = att * w  (mixed fp16*bf16, both 2-byte -> still 2x)
                tt = wpool.tile([R, N], BF16, tag="tt", bufs=8, name=f"tt{b}")
                nc.vector.tensor_mul(tt[:, :], att[:, :], w[:, :])
                yield

                # x.T accumulation in PSUM via identity matmuls:
                #   x = att + u - tt  =>  xT[c] = att_c.T + u_c.T + tt_c.T@(-I)
                psum_xt = pp_xt.tile([128, N], F32, tag="pxt", name=f"pxt{b}")
                for c in range(JC):
                    sl = slice(c * 128, (c + 1) * 128)
                    nc.tensor.matmul(psum_xt[:, sl], att[:, sl], idf[:, :],
                                     start=True, stop=False)
                    nc.tensor.matmul(psum_xt[:, sl], u[:, sl], idb[:, :],
                                     start=False, stop=False)
                    nc.tensor.matmul(psum_xt[:, sl], tt[:, sl], idbn[:, :],
                                     start=False, stop=True)
                yield

                # clip via two ACT relu passes: A' = 1.5 - y2
                y1 = wpool.tile([128, N], F32, tag="y1", name=f"y1{b}")
                nc.scalar.activation(y1[:, :], psum_xt[:, :], AFT.Relu,
                                     bias=half_col[:, :], scale=1.0)
                yield
                y2 = wpool.tile([128, N], F32R, tag="y2", bufs=3, name=f"y2{b}")
                nc.scalar.activation(y2[:, :], y1[:, :], AFT.Relu,
                                     bias=two_col[:, :], scale=-1.0)
                yield

                # out[i, e] = sum_j y2T[j, i] * (-vwT[j, e]) + bias'
                psum_o = pp_o.tile([R, E], F32, tag="po", name=f"po{b}")
                for c in range(JC):
                    nc.tensor.matmul(psum_o[:, :],
                                     y2[:, c * 128:(c + 1) * 128],
                                     vwTn[:, c * E:(c + 1) * E],
                                     start=(c == 0), stop=False)
                nc.tensor.matmul(psum_o[:, :], ones[:, :],
                                 vb_sb[:, :],
                                 start=False, stop=True)
                yield

                out_sb = wpool.tile([R, E], F32, tag="out_sb", name=f"osb{b}")
                nc.scalar.copy(out_sb[:, :], psum_o[:, :])
                nc.sync.dma_start(out_d[b, :, :], out_sb[:, :])
                yield

            GROUP = 2
            for g0 in range(0, B, GROUP):
                gens = [batch_chain(b) for b in range(g0, min(g0 + GROUP, B))]
                alive = list(gens)
                while alive:
                    for gen in list(alive):
                        try:
                            next(gen)
                        except StopIteration:
                            alive.remove(gen)

    nc.finalize()
    return nc


def get_nc():
    if "nc" not in _BUILD_CACHE:
        _BUILD_CACHE["nc"] = _build_nc()
    return _BUILD_CACHE["nc"]


def make_in_maps(inputs):
    spikes = np.asarray(inputs["spikes"])
    pre_trace = np.asarray(inputs["pre_trace"], dtype=np.float32)
    post_trace = np.asarray(inputs["post_trace"], dtype=np.float32)
    attention = np.asarray(inputs["attention"], dtype=np.float32)
    w_pre = np.asarray(inputs["latent_pre_weight"], dtype=np.float32)[0]
    w_post = np.asarray(inputs["latent_post_weight"], dtype=np.float32)[0]
    tau_pre = np.asarray(inputs["latent_pre_tau_s"], dtype=np.float32)[0]
    tau_post = np.asarray(inputs["latent_post_tau_s"], dtype=np.float32)[0]
    v_w = np.asarray(inputs["v_w"], dtype=np.float32)
    v_b = np.asarray(inputs["v_b"], dtype=np.float32)

    s = spikes.astype(np.float32)
    vwTn = np.ascontiguousarray(-v_w.T)          # [N, E], negated
    vbp = (v_b + 1.5 * v_w.sum(axis=1)).reshape(1, E).astype(np.float32)
    idf = np.eye(128, dtype=np.float16)
    idb = np.eye(128, dtype=ml_dtypes.bfloat16)

    bf = ml_dtypes.bfloat16
    sj_rep = np.ascontiguousarray(
        np.broadcast_to(s.astype(bf)[:, None, :], (B, R, N)))
    pre_bf = pre_trace.astype(bf)
    post_bf = post_trace.astype(bf)
    att_hf = attention.astype(np.float16)
    tau_pre_bf = tau_pre.astype(bf)
    tau_post_bf = tau_post.astype(bf)
    w_pre_bf = w_pre.astype(bf)
    w_post_bf = w_post.astype(bf)

    in_maps = []
    for c in range(NCORES):
        rows = slice(c * R, (c + 1) * R)
        pk = np.concatenate(
            [pre_bf[:, rows, :], post_bf[:, rows, :], sj_rep[:, :R, :]], axis=2)
        lat = np.concatenate(
            [tau_pre_bf[rows, :], tau_post_bf[rows, :],
             w_pre_bf[rows, :], w_post_bf[rows, :]], axis=1)
        in_maps.append({
            "pk": np.ascontiguousarray(pk),
            "att": np.ascontiguousarray(att_hf[:, rows, :]),
            "lat": np.ascontiguousarray(lat),
            "si": np.ascontiguousarray(s[:, rows].T),
            "vwTn": vwTn,
            "vb": vbp,
            "ones": np.ones((1, 128), dtype=np.float32),
            "idf": idf,
            "idb": idb,
            "idbn": np.ascontiguousarray(-idb),
        })
    return in_maps


def gather_out(results):
    out = np.empty((B, N, E), dtype=np.float32)
    for c in range(NCORES):
        out[:, c * R:(c + 1) * R, :] = results[c]["out"]
    return out


def run(inputs, trace=False, **kw):
    nc = get_nc()
    in_maps = make_in_maps(inputs)
    res = run_bass_kernel_spmd(nc, in_maps, list(range(NCORES)), trace=trace, **kw)
    return gather_out(res.results), res


def kernel(**inputs) -> np.ndarray:
    out, _ = run(inputs, trace=False)
    return out


# revision 42
# speedup vs baseline: 1.0210x; 1.0210x over previous
"""Trainium2 Bass kernel for nn_EphysAttentionLayer.

Reference semantics:
    s  = spikes.f32                              # [B, N] in {0,1}
    PD = exp(-DT / exp(tau_pre))                 # [N, N]
    QD = exp(-DT / exp(tau_post))
    pt' = pt*PD + s[b,j]*exp(w_pre)*DT
    qt' = qt*QD + s[b,i]*exp(w_post)*DT
    A'  = clip(att + (1-att)*pt'*si - att*qt'*sj, -0.5, 1.5)
    out = A' @ v_w.T + v_b                       # [B, N, E]

Sharding: rows (post-synaptic axis i) split across 8 cores, 128 rows each.
Per-core layout: [i on partitions, j in free dim], one batch at a time.

Key structure (per batch):
  u  = si * (PD*pt + SJ*preW')        (preW' = exp(w_pre + ln DT))
  q' = QD*qt + si*postW'              (post trace update)
  w  = u + SJ*q'
  d  = u - att*w                      (small delta, bf16)
  x  = att + d                        (accumulated transposed in PSUM via
                                       identity matmuls: x.T = att.T + d.T)
  A' = clip(x) = 1.5 - y2,  y2 = relu(2 - relu(x + 0.5))   (two ACT passes)
  out = y2 @ (-v_w.T) + (v_b + 1.5*rowsum(v_w))            (bias via K=1 MM)

dtypes: traces bf16, att fp16, latents bf16, compute chain bf16, x in fp32
PSUM; the output matmul runs in float32r (fast fp32 streaming mode).
The SJ broadcast masks and packed inputs are prepared host-side as part of
sharding; all O(N^2) compute stays on device.
"""

import math

import numpy as np
import ml_dtypes

import concourse.bacc as bacc
import concourse.mybir as mybir
import concourse.tile as tile
from concourse.bass_utils import run_bass_kernel_spmd

B, N, E = 8, 1024, 512
NCORES = 8
R = N // NCORES  # 128 rows per core
JC = N // 128    # 8 column chunks
DT = 0.001
LN_DT = math.log(DT)
MIN_ATTN, MAX_ATTN = -0.5, 1.5

F32 = mybir.dt.float32
F32R = mybir.dt.float32r
BF16 = mybir.dt.bfloat16
FP16 = mybir.dt.float16
AOP = mybir.AluOpType
AFT = mybir.ActivationFunctionType

_BUILD_CACHE = {}


def _build_nc():
    # Bacc (not raw Bass): its compile pipeline splits multi-sem waits into
    # InstEventSemaphore chains, which walrus codegen requires on TRN2.
    nc = bacc.Bacc()

    # pk: per-batch packed [pt | qt | SJ] along the free dim, bf16
    pk_d = nc.declare_dram_parameter("pk", [B, R, 3 * N], BF16, isOutput=False)
    att_d = nc.declare_dram_parameter("att", [B, R, N], FP16, isOutput=False)
    # lat: packed [tau_pre | tau_post | w_pre | w_post], bf16
    lat_d = nc.declare_dram_parameter("lat", [R, 4 * N], BF16, isOutput=False)
    si_d = nc.declare_dram_parameter("si", [R, B], F32, isOutput=False)
    vwTn_d = nc.declare_dram_parameter("vwTn", [N, E], F32R, isOutput=False)
    vb_d = nc.declare_dram_parameter("vb", [1, E], F32R, isOutput=False)
    ones_d = nc.declare_dram_parameter("ones", [1, 128], F32R, isOutput=False)
    idf_d = nc.declare_dram_parameter("idf", [128, 128], FP16, isOutput=False)
    idb_d = nc.declare_dram_parameter("idb", [128, 128], BF16, isOutput=False)
    idbn_d = nc.declare_dram_parameter("idbn", [128, 128], BF16, isOutput=False)
    out_d = nc.declare_dram_parameter("out", [B, R, E], F32, isOutput=True)

    with tile.TileContext(nc) as tc:
        with (
            tc.sbuf_pool(name="const", bufs=1) as cpool,
            tc.sbuf_pool(name="work", bufs=2) as wpool,
            tc.psum_pool(name="pxt_pool", bufs=3) as pp_xt,
            tc.psum_pool(name="po_pool", bufs=2) as pp_o,
        ):
            # ---- constants ----
            lndt_col = cpool.tile([128, 1], F32)
            nc.vector.memset(lndt_col[:, :], LN_DT)
            half_col = cpool.tile([128, 1], F32)
            nc.vector.memset(half_col[:, :], 0.5)
            two_col = cpool.tile([128, 1], F32)
            nc.vector.memset(two_col[:, :], 2.0)

            lat_sb = cpool.tile([R, 4 * N], BF16)
            nc.sync.dma_start(lat_sb[:, 0:N], lat_d[:, 0:N])
            nc.sync.dma_start(lat_sb[:, N:2 * N], lat_d[:, N:2 * N])
            nc.gpsimd.dma_start(lat_sb[:, 2 * N:4 * N], lat_d[:, 2 * N:4 * N])
            tau_pre = lat_sb[:, 0 * N:1 * N]
            tau_post = lat_sb[:, 1 * N:2 * N]
            w_pre = lat_sb[:, 2 * N:3 * N]
            w_post = lat_sb[:, 3 * N:4 * N]

            # e1 = exp(LN_DT - tau) = DT/exp(tau)  (ACT, one pass per tau)
            # PD = exp(-e1) ~= 1 - e1 + e1^2/2  (DVE Taylor; |e1|<=0.15 for
            # tau >= -5, so the cubic term <= 6e-4 -- below bf16 noise)
            e1p = cpool.tile([R, N], BF16)
            e1q = cpool.tile([R, N], BF16)
            PD = cpool.tile([R, N], BF16)
            QD = cpool.tile([R, N], BF16)
            preW = cpool.tile([R, N], BF16)
            postW = cpool.tile([R, N], BF16)
            nc.scalar.activation(e1p[:, :], tau_pre, AFT.Exp,
                                 bias=lndt_col[:, :], scale=-1.0)
            nc.scalar.activation(e1q[:, :], tau_post, AFT.Exp,
                                 bias=lndt_col[:, :], scale=-1.0)
            nc.scalar.activation(preW[:, :], w_pre, AFT.Exp,
                                 bias=lndt_col[:, :], scale=1.0)
            nc.scalar.activation(postW[:, :], w_post, AFT.Exp,
                                 bias=lndt_col[:, :], scale=1.0)
            gp = cpool.tile([R, N], BF16)
            gq = cpool.tile([R, N], BF16)
            nc.vector.tensor_scalar(gp[:, :], e1p[:, :], -0.5, 1.0, AOP.mult, AOP.add)
            nc.vector.tensor_scalar(gq[:, :], e1q[:, :], -0.5, 1.0, AOP.mult, AOP.add)
            nc.vector.tensor_mul(PD[:, :], e1p[:, :], gp[:, :])
            nc.vector.tensor_mul(QD[:, :], e1q[:, :], gq[:, :])
            nc.vector.tensor_scalar(PD[:, :], PD[:, :], -1.0, 1.0, AOP.mult, AOP.add)
            nc.vector.tensor_scalar(QD[:, :], QD[:, :], -1.0, 1.0, AOP.mult, AOP.add)

            # small consts: none are needed in the first ~10us; keep them off
            # the SP queue's head so vwTn and outputs aren't delayed
            si_sb = cpool.tile([R, B], F32)
            nc.sync.dma_start(si_sb[:, :], si_d[:, :])
            idf = cpool.tile([128, 128], FP16)
            nc.sync.dma_start(idf[:, :], idf_d[:, :])
            idb = cpool.tile([128, 128], BF16)
            nc.sync.dma_start(idb[:, :], idb_d[:, :])
            idbn = cpool.tile([128, 128], BF16)
            nc.sync.dma_start(idbn[:, :], idbn_d[:, :])
            vb_sb = cpool.tile([1, E], F32R)
            nc.sync.dma_start(vb_sb[:, :], vb_d[:, :])
            ones = cpool.tile([1, 128], F32R)
            nc.sync.dma_start(ones[:, :], ones_d[:, :])
            # vwTn DMA last: it is only needed by the first out-matmul (~15us
            # in) and must not delay the first batches' input DMAs.
            vwTn = cpool.tile([128, JC * E], F32R)  # chunk jc at [:, jc*E:(jc+1)*E]
            for jc in range(JC):
                nc.sync.dma_start(vwTn[:, jc * E:(jc + 1) * E],
                                  vwTn_d[jc * 128:(jc + 1) * 128, :])

            # ---- phase B: per-batch pipeline ----
            # Emitted as generators interleaved in pairs: consecutive DVE/ACT
            # instructions come from different batches, hiding the per-op
            # write-ack latency that would otherwise bubble dependent chains.

            def batch_chain(b):
                pk = wpool.tile([R, 3 * N], BF16, tag="pk", bufs=4, name=f"pk{b}")
                att = wpool.tile([R, N], FP16, tag="att", bufs=6, name=f"att{b}")
                nc.gpsimd.dma_start(pk[:, :], pk_d[b, :, :])
                nc.gpsimd.dma_start(att[:, :], att_d[b, :, :])
                pt = pk[:, 0 * N:1 * N]
                qt = pk[:, 1 * N:2 * N]
                SJ = pk[:, 2 * N:3 * N]
                si_b = si_sb[:, b:b + 1]
                yield

                # independent products first (DVE, bf16 2x)
                c1 = wpool.tile([R, N], BF16, tag="c1", bufs=3, name=f"c1{b}")
                nc.vector.tensor_mul(c1[:, :], PD[:, :], pt)
                yield
                m2 = wpool.tile([R, N], BF16, tag="m2", bufs=3, name=f"m2{b}")
                nc.vector.tensor_mul(m2[:, :], SJ, preW[:, :])
                yield
                a2 = wpool.tile([R, N], BF16, tag="a2", bufs=3, name=f"a2{b}")
                nc.vector.tensor_mul(a2[:, :], QD[:, :], qt)
                yield
                u0 = wpool.tile([R, N], BF16, tag="u0", bufs=4, name=f"u0{b}")
                nc.vector.tensor_add(u0[:, :], c1[:, :], m2[:, :])
                yield
                u = wpool.tile([R, N], BF16, tag="u", bufs=8, name=f"u{b}")
                nc.vector.tensor_scalar_mul(u[:, :], u0[:, :], si_b)
                yield
                m3 = wpool.tile([R, N], BF16, tag="m3", bufs=3, name=f"m3{b}")
                nc.vector.tensor_scalar_mul(m3[:, :], postW[:, :], si_b)
                yield
                v0 = wpool.tile([R, N], BF16, tag="v0", bufs=4, name=f"v0{b}")
                nc.vector.tensor_add(v0[:, :], a2[:, :], m3[:, :])
                yield
                vv = wpool.tile([R, N], BF16, tag="vv", bufs=3, name=f"vv{b}")
                nc.vector.tensor_mul(vv[:, :], SJ, v0[:, :])
                yield
                w = wpool.tile([R, N], BF16, tag="w", bufs=3, name=f"w{b}")
                nc.vector.tensor_add(w[:, :], u[:, :], vv[:, :])
                yield
                # tt = att * w  (mixed fp16*bf16, both 2-byte -> still 2x)
                tt = wpool.tile([R, N], BF16, tag="tt", bufs=8, name=f"tt{b}")
                nc.vector.tensor_mul(tt[:, :], att[:, :], w[:, :])
                yield

                # x.T accumulation in PSUM via identity matmuls:
                #   x = att + u - tt  =>  xT[c] = att_c.T + u_c.T + tt_c.T@(-I)
                psum_xt = pp_xt.tile([128, N], F32, tag="pxt", name=f"pxt{b}")
                for c in range(JC):
                    sl = slice(c * 128, (c + 1) * 128)
                    nc.tensor.matmul(psum_xt[:, sl], att[:, sl], idf[:, :],
                                     start=True, stop=False)
                    nc.tensor.matmul(psum_xt[:, sl], u[:, sl], idb[:, :],
                                     start=False, stop=False)
                    nc.tensor.matmul(psum_xt[:, sl], tt[:, sl], idbn[:, :],
                                     start=False, stop=True)
                yield

                # clip via two ACT relu passes: A' = 1.5 - y2
                y1 = wpool.tile([128, N], F32, tag="y1", bufs=3, name=f"y1{b}")
                nc.scalar.activation(y1[:, :], psum_xt[:, :], AFT.Relu,
                                     bias=half_col[:, :], scale=1.0)
                yield
                y2 = wpool.tile([128, N], F32R, tag="y2", bufs=3, name=f"y2{b}")
                nc.scalar.activation(y2[:, :], y1[:, :], AFT.Relu,
                                     bias=two_col[:, :], scale=-1.0)
                yield

                # out[i, e] = sum_j y2T[j, i] * (-vwT[j, e]) + bias'
                psum_o = pp_o.tile([R, E], F32, tag="po", name=f"po{b}")
                for c in range(JC):
                    nc.tensor.matmul(psum_o[:, :],
                                     y2[:, c * 128:(c + 1) * 128],
                                     vwTn[:, c * E:(c + 1) * E],
                                     start=(c == 0), stop=False)
                nc.tensor.matmul(psum_o[:, :], ones[:, :], vb_sb[:, :],
                                 start=False, stop=True)
                yield

                out_sb = wpool.tile([R, E], F32, tag="out_sb", name=f"osb{b}")
                nc.scalar.copy(out_sb[:, :], psum_o[:, :])
                nc.sync.dma_start(out_d[b, :, :], out_sb[:, :])
                yield

            GROUP = 2
            for g0 in range(0, B, GROUP):
                gens = [batch_chain(b) for b in range(g0, min(g0 + GROUP, B))]
                alive = list(gens)
                step = 0
                while alive:
                    for gen in list(alive):
                        try:
                            next(gen)
                        except StopIteration:
                            alive.remove(gen)
                    step += 1

    nc.finalize()
    return nc


def get_nc():
    if "nc" not in _BUILD_CACHE:
        _BUILD_CACHE["nc"] = _build_nc()
    return _BUILD_CACHE["nc"]


def make_in_maps(inputs):
    spikes = np.asarray(inputs["spikes"])
    pre_trace = np.asarray(inputs["pre_trace"], dtype=np.float32)
    post_trace = np.asarray(inputs["post_trace"], dtype=np.float32)
    attention = np.asarray(inputs["attention"], dtype=np.float32)
    w_pre = np.asarray(inputs["latent_pre_weight"], dtype=np.float32)[0]
    w_post = np.asarray(inputs["latent_post_weight"], dtype=np.float32)[0]
    tau_pre = np.asarray(inputs["latent_pre_tau_s"], dtype=np.float32)[0]
    tau_post = np.asarray(inputs["latent_post_tau_s"], dtype=np.float32)[0]
    v_w = np.asarray(inputs["v_w"], dtype=np.float32)
    v_b = np.asarray(inputs["v_b"], dtype=np.float32)

    s = spikes.astype(np.float32)
    vwTn = np.ascontiguousarray(-v_w.T)          # [N, E], negated
    vbp = (v_b + 1.5 * v_w.sum(axis=1)).reshape(1, E).astype(np.float32)
    idf = np.eye(128, dtype=np.float16)
    idb = np.eye(128, dtype=ml_dtypes.bfloat16)

    bf = ml_dtypes.bfloat16
    sj_rep = np.ascontiguousarray(
        np.broadcast_to(s.astype(bf)[:, None, :], (B, R, N)))
    pre_bf = pre_trace.astype(bf)
    post_bf = post_trace.astype(bf)
    att_hf = attention.astype(np.float16)
    tau_pre_bf = tau_pre.astype(bf)
    tau_post_bf = tau_post.astype(bf)
    w_pre_bf = w_pre.astype(bf)
    w_post_bf = w_post.astype(bf)

    in_maps = []
    for c in range(NCORES):
        rows = slice(c * R, (c + 1) * R)
        pk = np.concatenate(
            [pre_bf[:, rows, :], post_bf[:, rows, :], sj_rep[:, :R, :]], axis=2)
        lat = np.concatenate(
            [tau_pre_bf[rows, :], tau_post_bf[rows, :],
             w_pre_bf[rows, :], w_post_bf[rows, :]], axis=1)
        in_maps.append({
            "pk": np.ascontiguousarray(pk),
            "att": np.ascontiguousarray(att_hf[:, rows, :]),
            "lat": np.ascontiguousarray(lat),
            "si": np.ascontiguousarray(s[:, rows].T),
            "vwTn": vwTn,
            "vb": vbp,
            "ones": np.ones((1, 128), dtype=np.float32),
            "idf": idf,
            "idb": idb,
            "idbn": np.ascontiguousarray(-idb),
        })
    return in_maps


def gather_out(results):
    out = np.empty((B, N, E), dtype=np.float32)
    for c in range(NCORES):
        out[:, c * R:(c + 1) * R, :] = results[c]["out"]
    return out


def run(inputs, trace=False, **kw):
    nc = get_nc()
    in_maps = make_in_maps(inputs)
    res = run_bass_kernel_spmd(nc, in_maps, list(range(NCORES)), trace=trace, **kw)
    return gather_out(res.results), res


def kernel(**inputs) -> np.ndarray:
    out, _ = run(inputs, trace=False)
    return out


# revision 43
# speedup vs baseline: 1.0252x; 1.0040x over previous
"""Trainium2 Bass kernel for nn_EphysAttentionLayer.

Reference semantics:
    s  = spikes.f32                              # [B, N] in {0,1}
    PD = exp(-DT / exp(tau_pre))                 # [N, N]
    QD = exp(-DT / exp(tau_post))
    pt' = pt*PD + s[b,j]*exp(w_pre)*DT
    qt' = qt*QD + s[b,i]*exp(w_post)*DT
    A'  = clip(att + (1-att)*pt'*si - att*qt'*sj, -0.5, 1.5)
    out = A' @ v_w.T + v_b                       # [B, N, E]

Sharding: rows (post-synaptic axis i) split across 8 cores, 128 rows each.
Per-core layout: [i on partitions, j in free dim], one batch at a time.

Key structure (per batch):
  u  = si * (PD*pt + SJ*preW')        (preW' = exp(w_pre + ln DT))
  q' = QD*qt + si*postW'              (post trace update)
  w  = u + SJ*q'
  d  = u - att*w                      (small delta, bf16)
  x  = att + d                        (accumulated transposed in PSUM via
                                       identity matmuls: x.T = att.T + d.T)
  A' = clip(x) = 1.5 - y2,  y2 = relu(2 - relu(x + 0.5))   (two ACT passes)
  out = y2 @ (-v_w.T) + (v_b + 1.5*rowsum(v_w))            (bias via K=1 MM)

dtypes: traces bf16, att fp16, latents bf16, compute chain bf16, x in fp32
PSUM; the output matmul runs in float32r (fast fp32 streaming mode).
The SJ broadcast masks and packed inputs are prepared host-side as part of
sharding; all O(N^2) compute stays on device.
"""

import math

import numpy as np
import ml_dtypes

import concourse.bacc as bacc
import concourse.mybir as mybir
import concourse.tile as tile
from concourse.bass_utils import run_bass_kernel_spmd

B, N, E = 8, 1024, 512
NCORES = 8
R = N // NCORES  # 128 rows per core
JC = N // 128    # 8 column chunks
DT = 0.001
LN_DT = math.log(DT)
MIN_ATTN, MAX_ATTN = -0.5, 1.5

F32 = mybir.dt.float32
F32R = mybir.dt.float32r
BF16 = mybir.dt.bfloat16
FP16 = mybir.dt.float16
AOP = mybir.AluOpType
AFT = mybir.ActivationFunctionType

_BUILD_CACHE = {}


def _build_nc():
    # Bacc (not raw Bass): its compile pipeline splits multi-sem waits into
    # InstEventSemaphore chains, which walrus codegen requires on TRN2.
    nc = bacc.Bacc()

    # pk: per-batch packed [pt | qt | SJ] along the free dim, bf16
    pk_d = nc.declare_dram_parameter("pk", [B, R, 3 * N], BF16, isOutput=False)
    att_d = nc.declare_dram_parameter("att", [B, R, N], FP16, isOutput=False)
    # lat: packed [tau_pre | tau_post | w_pre | w_post], bf16
    lat_d = nc.declare_dram_parameter("lat", [R, 4 * N], BF16, isOutput=False)
    si_d = nc.declare_dram_parameter("si", [R, B], F32, isOutput=False)
    vwTn_d = nc.declare_dram_parameter("vwTn", [N, E], F32R, isOutput=False)
    vb_d = nc.declare_dram_parameter("vb", [1, E], F32R, isOutput=False)
    ones_d = nc.declare_dram_parameter("ones", [1, 128], F32R, isOutput=False)
    idf_d = nc.declare_dram_parameter("idf", [128, 128], FP16, isOutput=False)
    idb_d = nc.declare_dram_parameter("idb", [128, 128], BF16, isOutput=False)
    idbn_d = nc.declare_dram_parameter("idbn", [128, 128], BF16, isOutput=False)
    out_d = nc.declare_dram_parameter("out", [B, R, E], F32, isOutput=True)

    with tile.TileContext(nc) as tc:
        with (
            tc.sbuf_pool(name="const", bufs=1) as cpool,
            tc.sbuf_pool(name="work", bufs=2) as wpool,
            tc.psum_pool(name="pxt_pool", bufs=3) as pp_xt,
            tc.psum_pool(name="po_pool", bufs=2) as pp_o,
        ):
            # ---- constants ----
            lndt_col = cpool.tile([128, 1], F32)
            nc.vector.memset(lndt_col[:, :], LN_DT)
            half_col = cpool.tile([128, 1], F32)
            nc.vector.memset(half_col[:, :], 0.5)
            two_col = cpool.tile([128, 1], F32)
            nc.vector.memset(two_col[:, :], 2.0)

            lat_sb = cpool.tile([R, 4 * N], BF16)
            nc.sync.dma_start(lat_sb[:, 0:N], lat_d[:, 0:N])
            nc.sync.dma_start(lat_sb[:, N:2 * N], lat_d[:, N:2 * N])
            nc.gpsimd.dma_start(lat_sb[:, 2 * N:4 * N], lat_d[:, 2 * N:4 * N])
            tau_pre = lat_sb[:, 0 * N:1 * N]
            tau_post = lat_sb[:, 1 * N:2 * N]
            w_pre = lat_sb[:, 2 * N:3 * N]
            w_post = lat_sb[:, 3 * N:4 * N]

            # e1 = exp(LN_DT - tau) = DT/exp(tau)  (ACT, one pass per tau)
            # PD = exp(-e1) ~= 1 - e1 + e1^2/2  (DVE Taylor; |e1|<=0.15 for
            # tau >= -5, so the cubic term <= 6e-4 -- below bf16 noise)
            e1p = cpool.tile([R, N], BF16)
            e1q = cpool.tile([R, N], BF16)
            PD = cpool.tile([R, N], BF16)
            QD = cpool.tile([R, N], BF16)
            preW = cpool.tile([R, N], BF16)
            postW = cpool.tile([R, N], BF16)
            nc.scalar.activation(e1p[:, :], tau_pre, AFT.Exp,
                                 bias=lndt_col[:, :], scale=-1.0)
            nc.scalar.activation(e1q[:, :], tau_post, AFT.Exp,
                                 bias=lndt_col[:, :], scale=-1.0)
            nc.scalar.activation(preW[:, :], w_pre, AFT.Exp,
                                 bias=lndt_col[:, :], scale=1.0)
            nc.scalar.activation(postW[:, :], w_post, AFT.Exp,
                                 bias=lndt_col[:, :], scale=1.0)
            gp = cpool.tile([R, N], BF16)
            gq = cpool.tile([R, N], BF16)
            nc.vector.tensor_scalar(gp[:, :], e1p[:, :], -0.5, 1.0, AOP.mult, AOP.add)
            nc.vector.tensor_scalar(gq[:, :], e1q[:, :], -0.5, 1.0, AOP.mult, AOP.add)
            nc.vector.tensor_mul(PD[:, :], e1p[:, :], gp[:, :])
            nc.vector.tensor_mul(QD[:, :], e1q[:, :], gq[:, :])
            nc.vector.tensor_scalar(PD[:, :], PD[:, :], -1.0, 1.0, AOP.mult, AOP.add)
            nc.vector.tensor_scalar(QD[:, :], QD[:, :], -1.0, 1.0, AOP.mult, AOP.add)

            # small consts: none are needed in the first ~10us; keep them off
            # the SP queue's head so vwTn and outputs aren't delayed
            si_sb = cpool.tile([R, B], F32)
            nc.sync.dma_start(si_sb[:, :], si_d[:, :])
            idf = cpool.tile([128, 128], FP16)
            nc.sync.dma_start(idf[:, :], idf_d[:, :])
            idb = cpool.tile([128, 128], BF16)
            nc.sync.dma_start(idb[:, :], idb_d[:, :])
            idbn = cpool.tile([128, 128], BF16)
            nc.sync.dma_start(idbn[:, :], idbn_d[:, :])
            vb_sb = cpool.tile([1, E], F32R)
            nc.sync.dma_start(vb_sb[:, :], vb_d[:, :])
            ones = cpool.tile([1, 128], F32R)
            nc.sync.dma_start(ones[:, :], ones_d[:, :])
            # vwTn DMA last: it is only needed by the first out-matmul (~15us
            # in) and must not delay the first batches' input DMAs.
            vwTn = cpool.tile([128, JC * E], F32R)  # chunk jc at [:, jc*E:(jc+1)*E]
            for jc in range(JC):
                nc.sync.dma_start(vwTn[:, jc * E:(jc + 1) * E],
                                  vwTn_d[jc * 128:(jc + 1) * 128, :])

            # ---- phase B: per-batch pipeline ----
            # Emitted as generators interleaved in pairs: consecutive DVE/ACT
            # instructions come from different batches, hiding the per-op
            # write-ack latency that would otherwise bubble dependent chains.

            def batch_chain(b):
                pk = wpool.tile([R, 3 * N], BF16, tag="pk", bufs=4, name=f"pk{b}")
                att = wpool.tile([R, N], FP16, tag="att", bufs=6, name=f"att{b}")
                nc.gpsimd.dma_start(pk[:, :], pk_d[b, :, :])
                nc.gpsimd.dma_start(att[:, :], att_d[b, :, :])
                pt = pk[:, 0 * N:1 * N]
                qt = pk[:, 1 * N:2 * N]
                SJ = pk[:, 2 * N:3 * N]
                si_b = si_sb[:, b:b + 1]
                yield

                # independent products first (DVE, bf16 2x)
                c1 = wpool.tile([R, N], BF16, tag="c1", bufs=3, name=f"c1{b}")
                nc.vector.tensor_mul(c1[:, :], PD[:, :], pt)
                yield
                m2 = wpool.tile([R, N], BF16, tag="m2", bufs=3, name=f"m2{b}")
                nc.vector.tensor_mul(m2[:, :], SJ, preW[:, :])
                yield
                a2 = wpool.tile([R, N], BF16, tag="a2", bufs=3, name=f"a2{b}")
                nc.vector.tensor_mul(a2[:, :], QD[:, :], qt)
                yield
                u0 = wpool.tile([R, N], BF16, tag="u0", bufs=4, name=f"u0{b}")
                nc.vector.tensor_add(u0[:, :], c1[:, :], m2[:, :])
                yield
                u = wpool.tile([R, N], BF16, tag="u", bufs=8, name=f"u{b}")
                nc.vector.tensor_scalar_mul(u[:, :], u0[:, :], si_b)
                yield
                m3 = wpool.tile([R, N], BF16, tag="m3", bufs=3, name=f"m3{b}")
                nc.vector.tensor_scalar_mul(m3[:, :], postW[:, :], si_b)
                yield
                v0 = wpool.tile([R, N], BF16, tag="v0", bufs=4, name=f"v0{b}")
                nc.vector.tensor_add(v0[:, :], a2[:, :], m3[:, :])
                yield
                vv = wpool.tile([R, N], BF16, tag="vv", bufs=3, name=f"vv{b}")
                nc.vector.tensor_mul(vv[:, :], SJ, v0[:, :])
                yield
                w = wpool.tile([R, N], BF16, tag="w", bufs=3, name=f"w{b}")
                nc.vector.tensor_add(w[:, :], u[:, :], vv[:, :])
                yield
                # tt = att * w  (mixed fp16*bf16, both 2-byte -> still 2x)
                tt = wpool.tile([R, N], BF16, tag="tt", bufs=8, name=f"tt{b}")
                nc.vector.tensor_mul(tt[:, :], att[:, :], w[:, :])
                yield

                # x.T accumulation in PSUM via identity matmuls:
                #   x = att + u - tt  =>  xT[c] = att_c.T + u_c.T + tt_c.T@(-I)
                psum_xt = pp_xt.tile([128, N], F32, tag="pxt", name=f"pxt{b}")
                for c in range(JC):
                    sl = slice(c * 128, (c + 1) * 128)
                    nc.tensor.matmul(psum_xt[:, sl], att[:, sl], idf[:, :],
                                     start=True, stop=False)
                    nc.tensor.matmul(psum_xt[:, sl], u[:, sl], idb[:, :],
                                     start=False, stop=False)
                    nc.tensor.matmul(psum_xt[:, sl], tt[:, sl], idbn[:, :],
                                     start=False, stop=True)
                yield

                # clip via two ACT relu passes: A' = 1.5 - y2
                # (final batch: half-tile pipelining to shorten the drain)
                y1 = wpool.tile([128, N], F32, tag="y1", bufs=3, name=f"y1{b}")
                y2 = wpool.tile([128, N], F32R, tag="y2", bufs=3, name=f"y2{b}")
                psum_o = pp_o.tile([R, E], F32, tag="po", name=f"po{b}")
                halves = ((0, N // 2), (N // 2, N)) if b == B - 1 else ((0, N),)
                for (h0, h1) in halves:
                    nc.scalar.activation(y1[:, h0:h1], psum_xt[:, h0:h1], AFT.Relu,
                                         bias=half_col[:, :], scale=1.0)
                    yield
                    nc.scalar.activation(y2[:, h0:h1], y1[:, h0:h1], AFT.Relu,
                                         bias=two_col[:, :], scale=-1.0)
                    yield
                    for c in range(h0 // 128, h1 // 128):
                        nc.tensor.matmul(psum_o[:, :],
                                         y2[:, c * 128:(c + 1) * 128],
                                         vwTn[:, c * E:(c + 1) * E],
                                         start=(c == 0), stop=False)
                nc.tensor.matmul(psum_o[:, :], ones[:, :], vb_sb[:, :],
                                 start=False, stop=True)
                yield

                out_sb = wpool.tile([R, E], F32, tag="out_sb", name=f"osb{b}")
                nc.scalar.copy(out_sb[:, :], psum_o[:, :])
                nc.sync.dma_start(out_d[b, :, :], out_sb[:, :])
                yield

            GROUP = 2
            for g0 in range(0, B, GROUP):
                gens = [batch_chain(b) for b in range(g0, min(g0 + GROUP, B))]
                alive = list(gens)
                step = 0
                while alive:
                    for gen in list(alive):
                        try:
                            next(gen)
                        except StopIteration:
                            alive.remove(gen)
                    step += 1

    nc.finalize()
    return nc


def get_nc():
    if "nc" not in _BUILD_CACHE:
        _BUILD_CACHE["nc"] = _build_nc()
    return _BUILD_CACHE["nc"]


def make_in_maps(inputs):
    spikes = np.asarray(inputs["spikes"])
    pre_trace = np.asarray(inputs["pre_trace"], dtype=np.float32)
    post_trace = np.asarray(inputs["post_trace"], dtype=np.float32)
    attention = np.asarray(inputs["attention"], dtype=np.float32)
    w_pre = np.asarray(inputs["latent_pre_weight"], dtype=np.float32)[0]
    w_post = np.asarray(inputs["latent_post_weight"], dtype=np.float32)[0]
    tau_pre = np.asarray(inputs["latent_pre_tau_s"], dtype=np.float32)[0]
    tau_post = np.asarray(inputs["latent_post_tau_s"], dtype=np.float32)[0]
    v_w = np.asarray(inputs["v_w"], dtype=np.float32)
    v_b = np.asarray(inputs["v_b"], dtype=np.float32)

    s = spikes.astype(np.float32)
    vwTn = np.ascontiguousarray(-v_w.T)          # [N, E], negated
    vbp = (v_b + 1.5 * v_w.sum(axis=1)).reshape(1, E).astype(np.float32)
    idf = np.eye(128, dtype=np.float16)
    idb = np.eye(128, dtype=ml_dtypes.bfloat16)

    bf = ml_dtypes.bfloat16
    sj_rep = np.ascontiguousarray(
        np.broadcast_to(s.astype(bf)[:, None, :], (B, R, N)))
    pre_bf = pre_trace.astype(bf)
    post_bf = post_trace.astype(bf)
    att_hf = attention.astype(np.float16)
    tau_pre_bf = tau_pre.astype(bf)
    tau_post_bf = tau_post.astype(bf)
    w_pre_bf = w_pre.astype(bf)
    w_post_bf = w_post.astype(bf)

    in_maps = []
    for c in range(NCORES):
        rows = slice(c * R, (c + 1) * R)
        pk = np.concatenate(
            [pre_bf[:, rows, :], post_bf[:, rows, :], sj_rep[:, :R, :]], axis=2)
        lat = np.concatenate(
            [tau_pre_bf[rows, :], tau_post_bf[rows, :],
             w_pre_bf[rows, :], w_post_bf[rows, :]], axis=1)
        in_maps.append({
            "pk": np.ascontiguousarray(pk),
            "att": np.ascontiguousarray(att_hf[:, rows, :]),
            "lat": np.ascontiguousarray(lat),
            "si": np.ascontiguousarray(s[:, rows].T),
            "vwTn": vwTn,
            "vb": vbp,
            "ones": np.ones((1, 128), dtype=np.float32),
            "idf": idf,
            "idb": idb,
            "idbn": np.ascontiguousarray(-idb),
        })
    return in_maps


def gather_out(results):
    out = np.empty((B, N, E), dtype=np.float32)
    for c in range(NCORES):
        out[:, c * R:(c + 1) * R, :] = results[c]["out"]
    return out


def run(inputs, trace=False, **kw):
    nc = get_nc()
    in_maps = make_in_maps(inputs)
    res = run_bass_kernel_spmd(nc, in_maps, list(range(NCORES)), trace=trace, **kw)
    return gather_out(res.results), res


def kernel(**inputs) -> np.ndarray:
    out, _ = run(inputs, trace=False)
    return out
